# revision 1
# baseline (speedup 1.0000x reference)
"""Trainium2 Bass kernel for nn_MergeZoom: per-sample mask bbox + crop + bilinear resize.

Algorithm (per sample, all on-device):
  mb   = (mask >= 0.5)
  rows/cols nonzero -> bbox (first,last per axis) via exact count/weighted-sum trick
  out  = R @ (mb * image) @ C^T  where R/C are bilinear "tent" matrices built on-chip:
         R[ho, h] = relu(1 - |src_r(ho) - h|), src_r = clip(a*ho + b, lo, hi-1)
  Both interpolation stages are PE matmuls in bf16 (weights/data rounded; masks exact):
    stage1: T1t[w, ho] = (M_c)^T_as_lhsT . RT   (contracts h)
    stage2: out[ho,wo] = (T1t)^T_as_lhsT . CT   (contracts w)

Sharding: pure data-parallel, 4 samples per core across 8 cores.
"""

import numpy as np

import concourse.bass as bass
import concourse.tile as tile
from concourse import bacc, mybir

B = 32
N_CORES = 8
BPC = B // N_CORES  # samples per core
H = W = 512
C = 3
HT = H // 128  # 4 h-chunks of 128 partitions
WT = W // 128

FP = mybir.dt.float32
BF = mybir.dt.bfloat16
AX = mybir.AxisListType.X
OP = mybir.AluOpType
AF = mybir.ActivationFunctionType


def build(bpc: int = BPC) -> bass.Bass:
    nc = bacc.Bacc()
    mask_d = nc.declare_dram_parameter("mask", [bpc, H, W, 1], FP, isOutput=False)
    img_d = nc.declare_dram_parameter("image", [bpc, H, W, C], FP, isOutput=False)
    iota_d = nc.declare_dram_parameter("iota_f", [128, 512], FP, isOutput=False)
    pidx_d = nc.declare_dram_parameter("pidx", [128, HT], FP, isOutput=False)
    tp_d = nc.declare_dram_parameter("tp_h", [128, 2 * HT], BF, isOutput=False)
    out_d = nc.declare_dram_parameter("out", [bpc, H, W, C], FP, isOutput=True)

    with tile.TileContext(nc) as tc:
        with (
            tc.tile_pool(name="consts", bufs=1) as cpool,
            tc.tile_pool(name="io", bufs=2) as iopool,
            tc.tile_pool(name="work", bufs=1) as wk,
            tc.tile_pool(name="small", bufs=2) as sm,
            tc.tile_pool(name="ps1", bufs=2, space="PSUM") as ps1p,
            tc.tile_pool(name="ps2", bufs=2, space="PSUM") as ps2p,
            tc.tile_pool(name="psx", bufs=1, space="PSUM") as psxp,
        ):
            iota = cpool.tile([128, 512], FP)
            nc.sync.dma_start(iota[:], iota_d[:])
            pidx = cpool.tile([128, HT], FP)
            nc.sync.dma_start(pidx[:], pidx_d[:])
            tp = cpool.tile([128, 2 * HT], BF)
            nc.sync.dma_start(tp[:], tp_d[:])
            onesh = cpool.tile([128, 128], BF)
            nc.vector.memset(onesh[:], 1.0)
            negp = cpool.tile([128, HT], FP)
            nc.vector.tensor_scalar(negp[:], pidx[:], -1.0, None, OP.mult)

            for s in range(bpc):
                # ---------------- load ----------------
                msk = iopool.tile([128, HT * 512], FP, tag="msk")
                nc.sync.dma_start(
                    msk[:].rearrange("p (t w) -> p t w", t=HT),
                    mask_d[s]
                    .rearrange("(t p) w one -> t p (w one)", p=128)
                    .transpose([1, 0, 2]),
                )
                img = iopool.tile([128, HT * 512 * C], FP, tag="img")
                nc.sync.dma_start(
                    img[:].rearrange("p (t x) -> p t x", t=HT),
                    img_d[s]
                    .rearrange("(t p) w c -> t p (w c)", p=128)
                    .transpose([1, 0, 2]),
                )

                # ---------------- mask binarize (bf16) + row sums ----------------
                mbh = wk.tile([128, HT * 512], BF, tag="mbh", bufs=2)
                nc.vector.tensor_scalar(mbh[:], msk[:], 0.5, None, OP.is_ge)
                r4 = sm.tile([128, HT], FP, tag="r4")
                nc.vector.reduce_sum(
                    r4[:], mbh[:].rearrange("p (t w) -> p t w", t=HT), axis=AX
                )

                # masked image Mh = mb * image (bf16) on gpsimd (frees DVE)
                Mh = wk.tile([128, HT * 512 * C], BF, tag="Mh", bufs=2)
                img4 = img[:].rearrange("p (t w c) -> p t w c", t=HT, w=512)
                Mh4 = Mh[:].rearrange("p (t w c) -> p t w c", t=HT, w=512)
                mb3 = mbh[:].rearrange("p (t w) -> p t w", t=HT)
                for c in range(C):
                    nc.vector.tensor_tensor(
                        Mh4[:, :, :, c], img4[:, :, :, c], mb3, OP.mult
                    )

                # ---------------- col sums (replicated over partitions) ----------------
                pscols = psxp.tile([128, 512], FP, tag="pscols")
                for t in range(HT):
                    nc.tensor.matmul(
                        pscols[:],
                        onesh[:],
                        mbh[:, t * 512 : (t + 1) * 512],
                        start=(t == 0),
                        stop=(t == HT - 1),
                    )
                colnz = sm.tile([128, 512], FP, tag="colnz")
                nc.scalar.copy(colnz[:], pscols[:])

                # NS = [Nc, Sc, Nr, Sr_t, Sr_p] per partition (identical in all partitions)
                NS = sm.tile([128, 8], FP, tag="NS")
                nc.vector.tensor_scalar(colnz[:], colnz[:], 0.0, None, OP.is_gt)
                nc.vector.reduce_sum(NS[:, 0:1], colnz[:], axis=AX)
                nc.vector.tensor_tensor(colnz[:], colnz[:], iota[:], OP.mult)
                nc.vector.reduce_sum(NS[:, 1:2], colnz[:], axis=AX)

                # rows: rwh = [nz | nz*t | nz*p] in bf16; ones-matmul sums partitions
                rwh = sm.tile([128, 3 * HT], BF, tag="rwh")
                nc.vector.tensor_scalar(rwh[:, 0:HT], r4[:], 0.0, None, OP.is_gt)
                nc.vector.tensor_tensor(
                    rwh[:, HT : 2 * HT], rwh[:, 0:HT], tp[:, 0:HT], OP.mult
                )
                nc.vector.tensor_tensor(
                    rwh[:, 2 * HT : 3 * HT], rwh[:, 0:HT], tp[:, HT : 2 * HT], OP.mult
                )
                psrows = psxp.tile([128, 3 * HT], FP, tag="psrows")
                nc.tensor.matmul(psrows[:], onesh[:], rwh[:], start=True, stop=True)
                rsum = sm.tile([128, 3 * HT], FP, tag="rsum")
                nc.vector.tensor_copy(rsum[:], psrows[:])
                nc.vector.reduce_sum(NS[:, 2:3], rsum[:, 0:HT], axis=AX)
                nc.vector.reduce_sum(NS[:, 3:4], rsum[:, HT : 2 * HT], axis=AX)
                nc.vector.reduce_sum(NS[:, 4:5], rsum[:, 2 * HT : 3 * HT], axis=AX)
                # Sr = 128*Sr_t + Sr_p
                nc.vector.tensor_scalar(NS[:, 3:4], NS[:, 3:4], 128.0, None, OP.mult)
                nc.vector.tensor_tensor(NS[:, 3:4], NS[:, 3:4], NS[:, 4:5], OP.add)

                # ---------------- bbox scalars ----------------
                # mean = S/N; first = mean-(N-1)/2; last = mean+(N-1)/2
                # a = (last-first+2)/512 ; b = 0.5a - 1.5 + first ; lo = first-1 ; hi1 = last
                # sc layout per axis (rows at 0, cols at 8):
                #   0 recipN, 1 mean, 2 halfw, 3 first, 4 last(hi1), 5 a, 6 b, 7 lo
                sc = sm.tile([128, 16], FP, tag="sc")
                for n_ix, s_ix, o in ((2, 3, 0), (0, 1, 8)):
                    nc.vector.reciprocal(sc[:, o + 0 : o + 1], NS[:, n_ix : n_ix + 1])
                    nc.vector.tensor_tensor(
                        sc[:, o + 1 : o + 2], NS[:, s_ix : s_ix + 1],
                        sc[:, o + 0 : o + 1], OP.mult,
                    )
                    nc.vector.tensor_scalar(
                        sc[:, o + 2 : o + 3], NS[:, n_ix : n_ix + 1],
                        -1.0, 0.5, OP.add, OP.mult,
                    )
                    nc.vector.tensor_tensor(
                        sc[:, o + 3 : o + 4], sc[:, o + 1 : o + 2],
                        sc[:, o + 2 : o + 3], OP.subtract,
                    )
                    nc.vector.tensor_tensor(
                        sc[:, o + 4 : o + 5], sc[:, o + 1 : o + 2],
                        sc[:, o + 2 : o + 3], OP.add,
                    )
                    nc.vector.tensor_tensor(
                        sc[:, o + 5 : o + 6], sc[:, o + 4 : o + 5],
                        sc[:, o + 3 : o + 4], OP.subtract,
                    )
                    nc.vector.tensor_scalar(
                        sc[:, o + 5 : o + 6], sc[:, o + 5 : o + 6],
                        2.0, 1.0 / 512.0, OP.add, OP.mult,
                    )
                    nc.vector.tensor_scalar(
                        sc[:, o + 6 : o + 7], sc[:, o + 5 : o + 6],
                        0.5, -1.5, OP.mult, OP.add,
                    )
                    nc.vector.tensor_tensor(
                        sc[:, o + 6 : o + 7], sc[:, o + 6 : o + 7],
                        sc[:, o + 3 : o + 4], OP.add,
                    )
                    nc.vector.tensor_scalar(
                        sc[:, o + 7 : o + 8], sc[:, o + 3 : o + 4],
                        -1.0, None, OP.add,
                    )

                # ---------------- src vectors + tent matrices (bf16) ----------------
                RT = wk.tile([128, HT * 512], BF, tag="RT", bufs=2)
                CT = wk.tile([128, WT * 512], BF, tag="CT", bufs=2)
                for o, mat in ((0, RT), (8, CT)):
                    src = sm.tile([128, 512], FP, tag="src")
                    nc.vector.tensor_scalar(
                        src[:], iota[:], sc[:, o + 5 : o + 6], sc[:, o + 6 : o + 7],
                        OP.mult, OP.add,
                    )
                    nc.vector.tensor_scalar(
                        src[:], src[:], sc[:, o + 7 : o + 8], sc[:, o + 4 : o + 5],
                        OP.max, OP.min,
                    )
                    for t in range(HT):
                        tmp = sm.tile([128, 512], FP, tag="tenttmp")
                        nc.scalar.activation(
                            tmp[:], src[:], AF.Abs, bias=negp[:, t : t + 1], scale=1.0
                        )
                        nc.scalar.activation(
                            mat[:, t * 512 : (t + 1) * 512], tmp[:], AF.Relu,
                            bias=iota[:, 1:2], scale=-1.0,
                        )

                # ---------------- stage 1: T1t[w, ho] per channel ----------------
                t1 = wk.tile([128, C * WT * 512], BF, tag="t1")
                cp = 0
                for c in range(C):
                    for wt in range(WT):
                        ps1 = ps1p.tile([128, 512], FP, tag="ps1")
                        for ht in range(HT):
                            lhsT = Mh4[:, ht, wt * 128 : (wt + 1) * 128, c]
                            nc.tensor.matmul(
                                ps1[:],
                                lhsT,
                                RT[:, ht * 512 : (ht + 1) * 512],
                                start=(ht == 0),
                                stop=(ht == HT - 1),
                            )
                        dst = t1[:, (c * WT + wt) * 512 : (c * WT + wt + 1) * 512]
                        if cp % 2 == 0:
                            nc.scalar.copy(dst, ps1[:])
                        else:
                            nc.vector.tensor_copy(dst, ps1[:])
                        cp += 1

                # ---------------- stage 2 + output assembly ----------------
                outt = iopool.tile([128, HT * 512 * C], FP, tag="outt")
                out4 = outt[:].rearrange("p (t w c) -> p t w c", t=HT, w=512)
                for c in range(C):
                    for ot in range(HT):
                        ps2 = ps2p.tile([128, 512], FP, tag="ps2")
                        for wt in range(WT):
                            lhsT2 = t1[
                                :,
                                (c * WT + wt) * 512 + ot * 128 : (c * WT + wt) * 512
                                + (ot + 1) * 128,
                            ]
                            nc.tensor.matmul(
                                ps2[:],
                                lhsT2,
                                CT[:, wt * 512 : (wt + 1) * 512],
                                start=(wt == 0),
                                stop=(wt == WT - 1),
                            )
                        dst = out4[:, ot, :, c]
                        if cp % 2 == 0:
                            nc.scalar.copy(dst, ps2[:])
                        else:
                            nc.vector.tensor_copy(dst, ps2[:])
                        cp += 1

                nc.sync.dma_start(
                    out_d[s]
                    .rearrange("(t p) w c -> t p (w c)", p=128)
                    .transpose([1, 0, 2]),
                    outt[:].rearrange("p (t x) -> p t x", t=HT),
                )

    nc.compile()
    return nc


def make_consts() -> dict[str, np.ndarray]:
    import ml_dtypes

    iota_f = np.broadcast_to(np.arange(512, dtype=np.float32), (128, 512)).copy()
    # iota_f[:, 1] == 1.0 is used as the Relu bias constant
    p = np.arange(128, dtype=np.float32)
    pidx = np.stack([p + 128 * t for t in range(HT)], axis=1).astype(np.float32)
    tvals = np.broadcast_to(
        np.arange(HT, dtype=np.float32)[None, :], (128, HT)
    ).astype(np.float32)
    pvals = np.broadcast_to(p[:, None], (128, HT)).astype(np.float32)
    tp_h = np.concatenate([tvals, pvals], axis=1).astype(ml_dtypes.bfloat16)
    return {"iota_f": iota_f, "pidx": pidx, "tp_h": tp_h}


_NC_CACHE: dict[int, bass.Bass] = {}


def _get_nc(bpc: int = BPC) -> bass.Bass:
    if bpc not in _NC_CACHE:
        _NC_CACHE[bpc] = build(bpc)
    return _NC_CACHE[bpc]


def run(mask: np.ndarray, image: np.ndarray, trace: bool = False, **kwargs):
    """Run on 8 cores; returns (out [B,H,W,C], BassKernelResults)."""
    from concourse.bass_utils import run_bass_kernel_spmd

    nc = _get_nc(BPC)
    consts = make_consts()
    mask = np.ascontiguousarray(mask, dtype=np.float32)
    image = np.ascontiguousarray(image, dtype=np.float32)
    in_maps = []
    for i in range(N_CORES):
        m = {
            "mask": mask[i * BPC : (i + 1) * BPC],
            "image": image[i * BPC : (i + 1) * BPC],
        }
        m.update(consts)
        in_maps.append(m)
    res = run_bass_kernel_spmd(nc, in_maps, list(range(N_CORES)), trace=trace, **kwargs)
    out = np.concatenate([res.results[i]["out"] for i in range(N_CORES)], axis=0)
    return out, res


def kernel(mask: np.ndarray, image: np.ndarray) -> np.ndarray:
    out, _ = run(mask, image)
    return out.astype(np.float32)

